# revision 8
# baseline (speedup 1.0000x reference)
"""DecoderRNN Trainium2 kernel.

Math (reference):
    emb = embed_table[captions]                      # (B, 31, E)
    inputs = concat([features[:,None,:], emb], 1)    # (B, T=32, E)
    xproj = inputs @ Wi + bi                         # (B, T, H)
    h_t = tanh(xproj_t + h_{t-1} @ Wh + bh)          # scan over T
    out = hs @ Wy + by                               # (B, T, V)

Distribution: vocab-parallel output projection across 8 cores (Wy/by sharded
by 1250 columns); the embedding gather, input GEMM and serial RNN are
replicated full-batch on every core (they are cheap next to the projection
and the recurrence is inherently serial). No collectives.

On-chip layout: everything keeps H (or E) on the partition axis ("transposed"
activations), so the recurrence consumes and produces the same layout with no
per-step transposes:
    inputsT  [128, k, bt]    k = E/128 chunk, bt = t*64+b (t-major)
    xpT      [128, t, m, b]  m = H/128 chunk of the output
    hsT      [128, t, m, b]  t = 0..32 (slot 0 is h0 = 0)
Recurrence step (per output chunk m): psum[128,64] = sum_k Wh[k,m].T @ h[t-1,k]
with Wh tiles stationary; then DVE adds xpT, ACT applies tanh.

Dtypes: float32r for the big GEMMs (full PE rate at moving-dim >= 256,
~fp22 precision); fp16 for the recurrence weights/state (weight-load bound;
fp16 halves the per-matmul weight-load via fast-weight-load).
"""

import sys

sys.path.insert(0, "/opt/trn_rl_repo")

from contextlib import ExitStack

import numpy as np

import concourse.bass as bass
from concourse import bacc
import concourse.mybir as mybir
import concourse.tile as tile
from concourse.bass import ts
from concourse.bass_utils import run_bass_kernel_spmd

B, T, E, H, V = 64, 32, 512, 512, 10000
NCORES = 8
VS = V // NCORES          # vocab shard per core
BT = B * T                # 2048 rows, t-major: row = t*64 + b
P = 128
KE = E // P               # 4 contraction chunks over E
KH = H // P               # 4 contraction chunks over H
MT = H // P               # 4 output chunks of H
NBT = BT // P             # 16 bt tiles
F32 = mybir.dt.float32
MAIN = mybir.dt.float32r  # storage-identical to f32; PE truncates to ~fp22
RNN_DT = mybir.dt.float16

# projection N-chunks (all >= 256 so float32r runs at full rate; even sizes —
# fp32r streams element pairs and codegen rejects odd moving-row counts)
VCHUNKS = [(0, 418), (418, 416), (834, 416)]
assert sum(n for _, n in VCHUNKS) == VS


def build_program() -> bass.Bass:
    nc = bacc.Bacc()

    aug = nc.dram_tensor("aug_table", [V + B, E], MAIN, kind="ExternalInput")
    idx = nc.dram_tensor("idx", [P, NBT], mybir.dt.int32, kind="ExternalInput")
    wi = nc.dram_tensor("wi", [E, H], MAIN, kind="ExternalInput")
    wh = nc.dram_tensor("wh", [H, H], RNN_DT, kind="ExternalInput")
    bias = nc.dram_tensor("bias", [H], F32, kind="ExternalInput")  # bi + bh
    wy = nc.dram_tensor("wy", [H, VS], MAIN, kind="ExternalInput")
    by = nc.dram_tensor("by", [VS], F32, kind="ExternalInput")
    ident_d = nc.dram_tensor("ident", [P, P], MAIN, kind="ExternalInput")
    out = nc.dram_tensor("out", [BT, VS], F32, kind="ExternalOutput")

    with ExitStack() as ctx:
        tc = ctx.enter_context(tile.TileContext(nc))
        persist = ctx.enter_context(tc.tile_pool(name="persist", bufs=1))
        nat_pool = ctx.enter_context(tc.tile_pool(name="nat", bufs=3))
        z_pool = ctx.enter_context(tc.tile_pool(name="z", bufs=4))
        out_pool = ctx.enter_context(tc.tile_pool(name="outs", bufs=2))
        tp_psum = ctx.enter_context(tc.tile_pool(name="tp_ps", bufs=2, space="PSUM"))
        gemm_psum = ctx.enter_context(tc.tile_pool(name="gm_ps", bufs=3, space="PSUM"))
        rnn_psum = ctx.enter_context(tc.tile_pool(name="rn_ps", bufs=3, space="PSUM"))

        # ---- constants / weights into SBUF
        idx_sb = persist.tile([P, NBT], mybir.dt.int32, tag="idx")
        nc.sync.dma_start(out=idx_sb[:], in_=idx[:])

        ident = persist.tile([P, P], MAIN, tag="ident")
        nc.sync.dma_start(out=ident[:], in_=ident_d[:])
        identr = ident[:]

        bias_sb = persist.tile([P, MT], F32, tag="bias")
        nc.sync.dma_start(out=bias_sb[:], in_=bias[:].rearrange("(m p) -> p m", p=P))

        by_rep = persist.tile([P, VS], F32, tag="by_rep")
        by_bcast = bass.AP(tensor=by[:].tensor, offset=0, ap=[[0, P], [1, VS]])
        nc.gpsimd.dma_start(out=by_rep[:], in_=by_bcast)

        wi_sb = persist.tile([P, KE, H], MAIN, tag="wi")
        nc.sync.dma_start(out=wi_sb[:], in_=wi[:].rearrange("(k p) h -> p k h", p=P))

        wh_sb = persist.tile([P, KH, MT, P], RNN_DT, tag="wh")
        nc.sync.dma_start(
            out=wh_sb[:], in_=wh[:].rearrange("(k p) (m q) -> p k m q", p=P, q=P)
        )

        wy_sb = persist.tile([P, KH, VS], MAIN, tag="wy")
        nc.sync.dma_start(out=wy_sb[:], in_=wy[:].rearrange("(k p) v -> p k v", p=P))

        inputsT = persist.tile([P, KE, BT], MAIN, tag="inputsT")
        xpT = persist.tile([P, T, MT, B], MAIN, tag="xpT")
        # layout [P, m_or_k, (t, b)] so any (t-pair, k) slice is one
        # contiguous 128-wide free run (matmul stationary APs must be 1-D)
        hsT16 = persist.tile([P, MT, (T + 1) * B], RNN_DT, tag="hsT16")
        hsT32 = persist.tile([P, MT, (T + 1) * B], MAIN, tag="hsT32")
        nc.vector.memset(hsT16[:, :, 0:B], 0.0)

        # ---- gather embedding rows (+features appended as rows V..V+B) and
        # transpose to E-on-partitions
        for i in range(NBT):
            nat = nat_pool.tile([P, E], MAIN, tag="nat")
            nc.gpsimd.indirect_dma_start(
                out=nat[:],
                out_offset=None,
                in_=aug[:],
                in_offset=bass.IndirectOffsetOnAxis(ap=idx_sb[:, i : i + 1], axis=0),
            )
            for k in range(KE):
                pt = tp_psum.tile([P, P], MAIN, tag="tp")
                nc.tensor.transpose(pt[:], nat[:, ts(k, P)], identr)
                nc.scalar.activation(
                    inputsT[:, k, ts(i, P)], pt[:], mybir.ActivationFunctionType.Copy
                )

        # ---- xprojT = (inputs @ Wi).T + (bi + bh), stored [t, m, b]
        for m in range(MT):
            for nb in range(KE):  # 4 chunks of 512 bt-columns
                ps = gemm_psum.tile([P, 512], F32, tag="gemm")
                for k in range(KE):
                    nc.tensor.matmul(
                        ps[:],
                        lhsT=wi_sb[:, k, ts(m, P)],
                        rhs=inputsT[:, k, ts(nb, 512)],
                        start=(k == 0),
                        stop=(k == KE - 1),
                    )
                nc.scalar.activation(
                    xpT[:, 8 * nb : 8 * (nb + 1), m, :],
                    ps[:].rearrange("p (t b) -> p t b", b=B),
                    mybir.ActivationFunctionType.Identity,
                    bias=bias_sb[:, m : m + 1],
                )

        # ---- RNN: hsT[t] = tanh(xpT[t-1] + Wh.T-chunks @ hsT[t-1])
        for t in range(1, T + 1):
            for m in range(MT):
                rp = rnn_psum.tile([P, B], F32, tag="rnn")
                for k in range(KH):
                    nc.tensor.matmul(
                        rp[:],
                        lhsT=wh_sb[:, k, m, :],
                        rhs=hsT16[:, k, (t - 1) * B : t * B],
                        start=(k == 0),
                        stop=(k == KH - 1),
                    )
                z = z_pool.tile([P, B], F32, tag="z")
                nc.vector.tensor_add(z[:], rp[:], xpT[:, t - 1, m, :].bitcast(F32))
                nc.scalar.activation(
                    hsT16[:, m, t * B : (t + 1) * B],
                    z[:],
                    mybir.ActivationFunctionType.Tanh,
                )
                nc.scalar.activation(
                    hsT32[:, m, t * B : (t + 1) * B],
                    z[:],
                    mybir.ActivationFunctionType.Tanh,
                )

        # ---- projection: out[bt_tile] = hs @ Wy + by  (emitted after the RNN;
        # lower priority, so the scheduler uses it to fill PE gaps)
        for i in range(NBT):
            osb = out_pool.tile([P, VS], F32, tag="osb")
            for v0, vn in VCHUNKS:
                pp = gemm_psum.tile([P, 512], F32, tag="gemm")
                for k in range(KH):
                    nc.tensor.matmul(
                        pp[:, :vn],
                        lhsT=hsT32[:, k, (2 * i + 1) * B : (2 * i + 1) * B + P],
                        rhs=wy_sb[:, k, v0 : v0 + vn],
                        start=(k == 0),
                        stop=(k == KH - 1),
                    )
                nc.vector.tensor_add(
                    osb[:, v0 : v0 + vn], pp[:, :vn], by_rep[:, v0 : v0 + vn]
                )
            nc.sync.dma_start(out=out[ts(i, P), :], in_=osb[:])

    nc.compile()
    return nc


def make_in_maps(features, captions, embed_table, Wi, bi, Wh, bh, Wy, by):
    f32 = np.float32
    aug = np.concatenate(
        [np.asarray(embed_table, f32), np.asarray(features, f32)], axis=0
    )
    idx = np.empty((T, B), np.int32)
    idx[0] = V + np.arange(B, dtype=np.int32)
    idx[1:] = np.asarray(captions, np.int64).T.astype(np.int32)
    idx_t = np.ascontiguousarray(idx.reshape(BT).reshape(NBT, P).T)  # [128, 16]
    bias_c = (np.asarray(bi, f32) + np.asarray(bh, f32)).astype(f32)
    wi = np.ascontiguousarray(np.asarray(Wi, f32))
    wh16 = np.ascontiguousarray(np.asarray(Wh, f32).astype(np.float16))
    wy_f = np.asarray(Wy, f32)
    by_f = np.asarray(by, f32)
    in_maps = []
    for c in range(NCORES):
        in_maps.append(
            {
                "ident": np.eye(P, dtype=f32),
                "aug_table": aug,
                "idx": idx_t,
                "wi": wi,
                "wh": wh16,
                "bias": bias_c,
                "wy": np.ascontiguousarray(wy_f[:, c * VS : (c + 1) * VS]),
                "by": np.ascontiguousarray(by_f[c * VS : (c + 1) * VS]),
            }
        )
    return in_maps


def assemble(core_outs):
    full = np.concatenate(core_outs, axis=1)  # [BT, V]
    return np.ascontiguousarray(
        full.reshape(T, B, V).transpose(1, 0, 2).astype(np.float32)
    )


def kernel(**inputs) -> np.ndarray:
    in_maps = make_in_maps(**inputs)
    nc = build_program()
    res = run_bass_kernel_spmd(nc, in_maps, core_ids=list(range(NCORES)))
    return assemble([r["out"] for r in res.results])


# revision 10
# speedup vs baseline: 1.2056x; 1.2056x over previous
"""DecoderRNN Trainium2 kernel.

Math (reference):
    emb = embed_table[captions]                      # (B, 31, E)
    inputs = concat([features[:,None,:], emb], 1)    # (B, T=32, E)
    xproj = inputs @ Wi + bi                         # (B, T, H)
    h_t = tanh(xproj_t + h_{t-1} @ Wh + bh)          # scan over T
    out = hs @ Wy + by                               # (B, T, V)

Distribution: vocab-parallel output projection across 8 cores (Wy/by sharded
by 1250 columns); the embedding gather, input GEMM and serial RNN are
replicated full-batch on every core (they are cheap next to the projection
and the recurrence is inherently serial). No collectives.

On-chip layout: everything keeps H (or E) on the partition axis ("transposed"
activations), so the recurrence consumes and produces the same layout with no
per-step transposes:
    inputsT  [128, k, bt]     k = E/128 chunk, bt = t*64+b (t-major)
    xpT      [128, t, m*64+b] m = H/128 chunk of the output
    hsT      [128, m, t*64+b] t = 0..32 (slot 0 is h0 = 0)
Recurrence step: one [128,256] psum accumulates all 16 Wh-tile matmuls
(Wh stationary, h_{t-1} moving), then a single DVE add of xpT and a single
tanh per step.

All matmul operands are fp16: fp32 matmuls on trn2 run in the slow
fp32-HIGH mode AND disable fast-weight-load for neighbouring fp16 matmuls;
fp16 keeps the PE at 1 cycle/row with ~53ns weight loads. Accumulation is
fp32 in PSUM, bias adds are fp32. Measured end-to-end relative error vs the
fp32 reference is a few 1e-4.

The input embedding transpose (bt-major gather -> E-on-partitions) is done by
the DMA xbar: gather rows to SBUF, write a [2048, 512] fp16 DRAM scratch,
then four [2048,128] -> [128,2048] transpose-DMA reads. Zero PE/ACT cost.
"""

import sys

sys.path.insert(0, "/opt/trn_rl_repo")

from contextlib import ExitStack

import numpy as np

import concourse.bass as bass
import concourse.mybir as mybir
import concourse.tile as tile
from concourse import bacc
from concourse.bass import ts
from concourse.bass_utils import run_bass_kernel_spmd

B, T, E, H, V = 64, 32, 512, 512, 10000
NCORES = 8
VS = V // NCORES          # vocab shard per core
BT = B * T                # 2048 rows, t-major: row = t*64 + b
P = 128
KE = E // P               # 4 contraction chunks over E
KH = H // P               # 4 contraction chunks over H
MT = H // P               # 4 output chunks of H
NBT = BT // P             # 16 bt tiles
F32 = mybir.dt.float32
F16 = mybir.dt.float16

# projection N-chunks (fp16 streams 1 cycle/row at any size; <=512 per bank)
VCHUNKS = [(0, 512), (512, 512), (1024, 226)]
assert sum(n for _, n in VCHUNKS) == VS


def build_program() -> bass.Bass:
    nc = bacc.Bacc()

    aug = nc.dram_tensor("aug_table", [V + B, E], F16, kind="ExternalInput")
    idx = nc.dram_tensor("idx", [P, NBT], mybir.dt.int32, kind="ExternalInput")
    wi = nc.dram_tensor("wi", [E, H], F16, kind="ExternalInput")
    wh = nc.dram_tensor("wh", [H, H], F16, kind="ExternalInput")
    bias = nc.dram_tensor("bias", [H], F32, kind="ExternalInput")  # bi + bh
    wy = nc.dram_tensor("wy", [H, VS], F16, kind="ExternalInput")
    by = nc.dram_tensor("by", [VS], F32, kind="ExternalInput")
    out = nc.dram_tensor("out", [BT, VS], F32, kind="ExternalOutput")
    scratch = nc.dram_tensor("scratch", [BT, E], F16)  # gather staging

    with ExitStack() as ctx:
        tc = ctx.enter_context(tile.TileContext(nc))
        persist = ctx.enter_context(tc.tile_pool(name="persist", bufs=1))
        z_pool = ctx.enter_context(tc.tile_pool(name="z", bufs=3))
        out_pool = ctx.enter_context(tc.tile_pool(name="outs", bufs=2))
        gemm_psum = ctx.enter_context(tc.tile_pool(name="gm_ps", bufs=3, space="PSUM"))
        rnn_psum = ctx.enter_context(tc.tile_pool(name="rn_ps", bufs=2, space="PSUM"))

        # ---- gather first (it gates everything downstream)
        idx_sb = persist.tile([P, NBT], mybir.dt.int32, tag="idx")
        nc.sync.dma_start(out=idx_sb[:], in_=idx[:])

        nat = persist.tile([P, NBT, E], F16, tag="nat")
        for i in range(NBT):
            nc.gpsimd.indirect_dma_start(
                out=nat[:, i, :],
                out_offset=None,
                in_=aug[:],
                in_offset=bass.IndirectOffsetOnAxis(ap=idx_sb[:, i : i + 1], axis=0),
            )
            nc.sync.dma_start(out=scratch[ts(i, P), :], in_=nat[:, i, :])

        inputsT = persist.tile([P, KE, BT], F16, tag="inputsT")
        for k in range(KE):
            nc.sync.dma_start_transpose(
                out=inputsT[:, k, :], in_=scratch[:, ts(k, P)]
            )

        # ---- weights / constants
        wi_sb = persist.tile([P, KE, H], F16, tag="wi")
        nc.sync.dma_start(out=wi_sb[:], in_=wi[:].rearrange("(k p) h -> p k h", p=P))

        wh_sb = persist.tile([P, KH, MT, P], F16, tag="wh")
        nc.sync.dma_start(
            out=wh_sb[:], in_=wh[:].rearrange("(k p) (m q) -> p k m q", p=P, q=P)
        )

        bias_sb = persist.tile([P, MT], F32, tag="bias")
        nc.sync.dma_start(out=bias_sb[:], in_=bias[:].rearrange("(m p) -> p m", p=P))

        wy_sb = persist.tile([P, KH, VS], F16, tag="wy")
        nc.sync.dma_start(out=wy_sb[:], in_=wy[:].rearrange("(k p) v -> p k v", p=P))

        by_rep = persist.tile([P, VS], F32, tag="by_rep")
        by_bcast = bass.AP(tensor=by[:].tensor, offset=0, ap=[[0, P], [1, VS]])
        nc.gpsimd.dma_start(out=by_rep[:], in_=by_bcast)

        xpT = persist.tile([P, T, MT * B], F32, tag="xpT")
        hsT = persist.tile([P, MT, (T + 1) * B], F16, tag="hsT")
        nc.vector.memset(hsT[:, :, 0:B], 0.0)

        # ---- xprojT = (inputs @ Wi).T + (bi + bh), bt-chunk outer so the RNN
        # can start as soon as the first 8 timesteps are projected
        for nb in range(KE):  # 4 chunks of 512 bt-columns
            for m in range(MT):
                ps = gemm_psum.tile([P, 512], F32, tag="gemm")
                for k in range(KE):
                    nc.tensor.matmul(
                        ps[:],
                        lhsT=wi_sb[:, k, ts(m, P)],
                        rhs=inputsT[:, k, ts(nb, 512)],
                        start=(k == 0),
                        stop=(k == KE - 1),
                    )
                nc.scalar.activation(
                    xpT[:, 8 * nb : 8 * (nb + 1), ts(m, B)],
                    ps[:].rearrange("p (t b) -> p t b", b=B),
                    mybir.ActivationFunctionType.Identity,
                    bias=bias_sb[:, m : m + 1],
                )

        # ---- RNN: hsT[t] = tanh(xpT[t-1] + Wh.T-chunks @ hsT[t-1])
        for t in range(1, T + 1):
            rp = rnn_psum.tile([P, MT * B], F32, tag="rnn")
            for m in range(MT):
                for k in range(KH):
                    nc.tensor.matmul(
                        rp[:, ts(m, B)],
                        lhsT=wh_sb[:, k, m, :],
                        rhs=hsT[:, k, (t - 1) * B : t * B],
                        start=(k == 0),
                        stop=(k == KH - 1),
                    )
            z = z_pool.tile([P, MT * B], F32, tag="z")
            nc.vector.tensor_add(z[:], rp[:], xpT[:, t - 1, :])
            nc.scalar.activation(
                hsT[:, :, t * B : (t + 1) * B],
                z[:].rearrange("p (m b) -> p m b", b=B),
                mybir.ActivationFunctionType.Tanh,
            )

        # ---- projection: out[bt_tile] = hs @ Wy + by  (emitted after the RNN;
        # lower priority, so the scheduler uses it to fill PE gaps)
        for i in range(NBT):
            osb = out_pool.tile([P, VS], F32, tag="osb")
            for v0, vn in VCHUNKS:
                pp = gemm_psum.tile([P, 512], F32, tag="gemm")
                for k in range(KH):
                    nc.tensor.matmul(
                        pp[:, :vn],
                        lhsT=hsT[:, k, (2 * i + 1) * B : (2 * i + 1) * B + P],
                        rhs=wy_sb[:, k, v0 : v0 + vn],
                        start=(k == 0),
                        stop=(k == KH - 1),
                    )
                nc.vector.tensor_add(
                    osb[:, v0 : v0 + vn], pp[:, :vn], by_rep[:, v0 : v0 + vn]
                )
            nc.sync.dma_start(out=out[ts(i, P), :], in_=osb[:])

    nc.compile()
    return nc


def make_in_maps(features, captions, embed_table, Wi, bi, Wh, bh, Wy, by):
    f32, f16 = np.float32, np.float16
    aug = np.concatenate(
        [np.asarray(embed_table, f32), np.asarray(features, f32)], axis=0
    ).astype(f16)
    idx = np.empty((T, B), np.int32)
    idx[0] = V + np.arange(B, dtype=np.int32)
    idx[1:] = np.asarray(captions, np.int64).T.astype(np.int32)
    idx_t = np.ascontiguousarray(idx.reshape(BT).reshape(NBT, P).T)  # [128, 16]
    bias_c = (np.asarray(bi, f32) + np.asarray(bh, f32)).astype(f32)
    wi16 = np.ascontiguousarray(np.asarray(Wi, f32).astype(f16))
    wh16 = np.ascontiguousarray(np.asarray(Wh, f32).astype(f16))
    wy16 = np.asarray(Wy, f32).astype(f16)
    by_f = np.asarray(by, f32)
    in_maps = []
    for c in range(NCORES):
        in_maps.append(
            {
                "aug_table": aug,
                "idx": idx_t,
                "wi": wi16,
                "wh": wh16,
                "bias": bias_c,
                "wy": np.ascontiguousarray(wy16[:, c * VS : (c + 1) * VS]),
                "by": np.ascontiguousarray(by_f[c * VS : (c + 1) * VS]),
            }
        )
    return in_maps


def assemble(core_outs):
    full = np.concatenate(core_outs, axis=1)  # [BT, V]
    return np.ascontiguousarray(
        full.reshape(T, B, V).transpose(1, 0, 2).astype(np.float32)
    )


def kernel(**inputs) -> np.ndarray:
    in_maps = make_in_maps(**inputs)
    nc = build_program()
    res = run_bass_kernel_spmd(nc, in_maps, core_ids=list(range(NCORES)))
    return assemble([r["out"] for r in res.results])


# revision 14
# speedup vs baseline: 1.2869x; 1.0675x over previous
"""DecoderRNN Trainium2 kernel.

Math (reference):
    emb = embed_table[captions]                      # (B, 31, E)
    inputs = concat([features[:,None,:], emb], 1)    # (B, T=32, E)
    xproj = inputs @ Wi + bi                         # (B, T, H)
    h_t = tanh(xproj_t + h_{t-1} @ Wh + bh)          # scan over T
    out = hs @ Wy + by                               # (B, T, V)

Distribution: vocab-parallel output projection across 8 cores (Wy/by sharded
by 1250 columns); the embedding gather, input GEMM and serial RNN are
replicated full-batch on every core (they are cheap next to the projection
and the recurrence is inherently serial). No collectives.

On-chip layout: everything keeps H (or E) on the partition axis ("transposed"
activations), so the recurrence consumes and produces the same layout with no
per-step transposes:
    inputsT  [128, k, bt]     k = E/128 chunk, bt = t*64+b (t-major)
    xpT      [128, t, m*64+b] m = H/128 chunk of the output
    hsT      [128, m, t*64+b] t = 0..32 (slot 0 is h0 = 0)
Recurrence step: one [128,256] psum accumulates all 16 Wh-tile matmuls
(Wh stationary, h_{t-1} moving), then a single DVE add of xpT and a single
tanh per step.

All matmul operands are fp16: fp32 matmuls on trn2 run in the slow
fp32-HIGH mode AND disable fast-weight-load for neighbouring fp16 matmuls;
fp16 keeps the PE at 1 cycle/row with ~53ns weight loads. Accumulation is
fp32 in PSUM, bias adds are fp32. Measured end-to-end relative error vs the
fp32 reference is a few 1e-4.

The input embedding transpose (bt-major gather -> E-on-partitions) is done by
the DMA xbar: gather rows to SBUF, write a [2048, 512] fp16 DRAM scratch,
then four [2048,128] -> [128,2048] transpose-DMA reads. Zero PE/ACT cost.
"""

import sys

sys.path.insert(0, "/opt/trn_rl_repo")

from contextlib import ExitStack

import numpy as np

import concourse.bass as bass
import concourse.mybir as mybir
import concourse.tile as tile
from concourse import bacc
from concourse.bass import ts
from concourse.bass_utils import run_bass_kernel_spmd

B, T, E, H, V = 64, 32, 512, 512, 10000
NCORES = 8
VS = V // NCORES          # vocab shard per core
BT = B * T                # 2048 rows, t-major: row = t*64 + b
P = 128
KE = E // P               # 4 contraction chunks over E
KH = H // P               # 4 contraction chunks over H
MT = H // P               # 4 output chunks of H
NBT = BT // P             # 16 bt tiles
F32 = mybir.dt.float32
F16 = mybir.dt.float16

# projection N-chunks (fp16 streams 1 cycle/row at any size; <=512 per bank)
VCHUNKS = [(0, 512), (512, 512), (1024, 226)]
assert sum(n for _, n in VCHUNKS) == VS


def build_program() -> bass.Bass:
    nc = bacc.Bacc()

    aug = nc.dram_tensor("aug_table", [V + B, E], F16, kind="ExternalInput")
    idx = nc.dram_tensor("idx", [P, NBT], mybir.dt.int32, kind="ExternalInput")
    wi = nc.dram_tensor("wi", [E, H], F16, kind="ExternalInput")
    wh = nc.dram_tensor("wh", [H, H], F16, kind="ExternalInput")
    bias = nc.dram_tensor("bias", [H], F32, kind="ExternalInput")  # bi + bh
    wy = nc.dram_tensor("wy", [H, VS], F16, kind="ExternalInput")
    by = nc.dram_tensor("by", [VS], F32, kind="ExternalInput")
    out = nc.dram_tensor("out", [BT, VS], F32, kind="ExternalOutput")
    scratch = nc.dram_tensor("scratch", [BT, E], F16)  # gather staging

    with ExitStack() as ctx:
        tc = ctx.enter_context(tile.TileContext(nc))
        persist = ctx.enter_context(tc.tile_pool(name="persist", bufs=1))
        z_pool = ctx.enter_context(tc.tile_pool(name="z", bufs=3))
        out_pool = ctx.enter_context(tc.tile_pool(name="outs", bufs=2))
        gemm_psum = ctx.enter_context(tc.tile_pool(name="gm_ps", bufs=3, space="PSUM"))
        rnn_psum = ctx.enter_context(tc.tile_pool(name="rn_ps", bufs=2, space="PSUM"))

        # ---- gather first (it gates everything downstream)
        idx_sb = persist.tile([P, NBT], mybir.dt.int32, tag="idx")
        nc.sync.dma_start(out=idx_sb[:], in_=idx[:])

        nat = persist.tile([P, NBT, E], F16, tag="nat")
        for i in range(NBT):
            nc.gpsimd.indirect_dma_start(
                out=nat[:, i, :],
                out_offset=None,
                in_=aug[:],
                in_offset=bass.IndirectOffsetOnAxis(ap=idx_sb[:, i : i + 1], axis=0),
            )
            nc.sync.dma_start(out=scratch[ts(i, P), :], in_=nat[:, i, :])

        inputsT = persist.tile([P, KE, BT], F16, tag="inputsT")
        for k in range(KE):
            nc.sync.dma_start_transpose(
                out=inputsT[:, k, :], in_=scratch[:, ts(k, P)]
            )

        # ---- weights / constants (wy/by only needed by the projection, late)
        wi_sb = persist.tile([P, KE, H], F16, tag="wi")
        nc.sync.dma_start(out=wi_sb[:], in_=wi[:].rearrange("(k p) h -> p k h", p=P))

        wh_sb = persist.tile([P, KH, MT, P], F16, tag="wh")
        nc.sync.dma_start(
            out=wh_sb[:], in_=wh[:].rearrange("(k p) (m q) -> p k m q", p=P, q=P)
        )

        bias_sb = persist.tile([P, MT], F32, tag="bias")
        nc.sync.dma_start(out=bias_sb[:], in_=bias[:].rearrange("(m p) -> p m", p=P))

        xpT = persist.tile([P, T, MT * B], F32, tag="xpT")
        hsT = persist.tile([P, MT, (T + 1) * B], F16, tag="hsT")
        nc.vector.memset(hsT[:, :, 0:B], 0.0)

        # ---- xprojT = (inputs @ Wi).T + (bi + bh), bt-chunk outer so the RNN
        # can start as soon as the first 8 timesteps are projected
        for nb in range(KE):  # 4 chunks of 512 bt-columns
            for m in range(MT):
                ps = gemm_psum.tile([P, 512], F32, tag="gemm")
                for k in range(KE):
                    nc.tensor.matmul(
                        ps[:],
                        lhsT=wi_sb[:, k, ts(m, P)],
                        rhs=inputsT[:, k, ts(nb, 512)],
                        start=(k == 0),
                        stop=(k == KE - 1),
                    )
                nc.scalar.activation(
                    xpT[:, 8 * nb : 8 * (nb + 1), ts(m, B)],
                    ps[:].rearrange("p (t b) -> p t b", b=B),
                    mybir.ActivationFunctionType.Identity,
                    bias=bias_sb[:, m : m + 1],
                )


        # projection weights arrive mid-kernel, well before the projection
        wy_sb = persist.tile([P, KH, VS], F16, tag="wy")
        nc.sync.dma_start(out=wy_sb[:], in_=wy[:].rearrange("(k p) v -> p k v", p=P))

        by_rep = persist.tile([P, VS], F32, tag="by_rep")
        by_bcast = bass.AP(tensor=by[:].tensor, offset=0, ap=[[0, P], [1, VS]])
        nc.gpsimd.dma_start(out=by_rep[:], in_=by_bcast)

        # ---- RNN: hsT[t] = tanh(xpT[t-1] + Wh.T-chunks @ hsT[t-1])
        for t in range(1, T + 1):
            rp = rnn_psum.tile([P, MT * B], F32, tag="rnn")
            for m in range(MT):
                for k in range(KH):
                    nc.tensor.matmul(
                        rp[:, ts(m, B)],
                        lhsT=wh_sb[:, k, m, :],
                        rhs=hsT[:, k, (t - 1) * B : t * B],
                        start=(k == 0),
                        stop=(k == KH - 1),
                    )
            z = z_pool.tile([P, MT * B], F32, tag="z")
            nc.vector.tensor_add(z[:], rp[:], xpT[:, t - 1, :])
            nc.scalar.activation(
                hsT[:, :, t * B : (t + 1) * B],
                z[:].rearrange("p (m b) -> p m b", b=B),
                mybir.ActivationFunctionType.Tanh,
            )

        # ---- projection: out[bt_tile] = hs @ Wy + by  (emitted after the RNN;
        # lower priority, so the scheduler uses it to fill PE gaps)
        for i in range(NBT):
            osb = out_pool.tile([P, VS], F32, tag="osb")
            for v0, vn in VCHUNKS:
                pp = gemm_psum.tile([P, 512], F32, tag="gemm")
                for k in range(KH):
                    nc.tensor.matmul(
                        pp[:, :vn],
                        lhsT=hsT[:, k, (2 * i + 1) * B : (2 * i + 1) * B + P],
                        rhs=wy_sb[:, k, v0 : v0 + vn],
                        start=(k == 0),
                        stop=(k == KH - 1),
                    )
                nc.vector.tensor_add(
                    osb[:, v0 : v0 + vn], pp[:, :vn], by_rep[:, v0 : v0 + vn]
                )
            nc.sync.dma_start(out=out[ts(i, P), :], in_=osb[:])

    nc.compile()
    return nc


def make_in_maps(features, captions, embed_table, Wi, bi, Wh, bh, Wy, by):
    f32, f16 = np.float32, np.float16
    aug = np.concatenate(
        [np.asarray(embed_table, f32), np.asarray(features, f32)], axis=0
    ).astype(f16)
    idx = np.empty((T, B), np.int32)
    idx[0] = V + np.arange(B, dtype=np.int32)
    idx[1:] = np.asarray(captions, np.int64).T.astype(np.int32)
    idx_t = np.ascontiguousarray(idx.reshape(BT).reshape(NBT, P).T)  # [128, 16]
    bias_c = (np.asarray(bi, f32) + np.asarray(bh, f32)).astype(f32)
    wi16 = np.ascontiguousarray(np.asarray(Wi, f32).astype(f16))
    wh16 = np.ascontiguousarray(np.asarray(Wh, f32).astype(f16))
    wy16 = np.asarray(Wy, f32).astype(f16)
    by_f = np.asarray(by, f32)
    in_maps = []
    for c in range(NCORES):
        in_maps.append(
            {
                "aug_table": aug,
                "idx": idx_t,
                "wi": wi16,
                "wh": wh16,
                "bias": bias_c,
                "wy": np.ascontiguousarray(wy16[:, c * VS : (c + 1) * VS]),
                "by": np.ascontiguousarray(by_f[c * VS : (c + 1) * VS]),
            }
        )
    return in_maps


def assemble(core_outs):
    full = np.concatenate(core_outs, axis=1)  # [BT, V]
    return np.ascontiguousarray(
        full.reshape(T, B, V).transpose(1, 0, 2).astype(np.float32)
    )


def kernel(**inputs) -> np.ndarray:
    in_maps = make_in_maps(**inputs)
    nc = build_program()
    res = run_bass_kernel_spmd(nc, in_maps, core_ids=list(range(NCORES)))
    return assemble([r["out"] for r in res.results])
